# revision 1
# baseline (speedup 1.0000x reference)
"""DCN cross-layer kernel for Trainium2 (8 NeuronCores, data-parallel).

Reference computes, for i = 0..L-1:
    x_{i+1} = x0 * (x_i . w_i) + b_i + x_i         (x0 fixed, per-row dot)

Algebraic collapse: every iterate has the form x_i = alpha_i * x0 + beta_i
with per-row scalar alpha_i and a row-independent vector beta_i:
    alpha_0 = 1,  beta_0 = 0
    alpha_{i+1} = alpha_i * (1 + c_i) + gamma_i,   c_i = x0 . w_i (per row)
    beta_{i+1}  = beta_i + b_i,                    gamma_i = beta_i . w_i
    out = alpha_L * x0 + beta_L

So the whole module reduces to one skinny matmul C = x0 @ W^T (B x L), a
tiny per-row recurrence over L=4 scalars, and one fused scale-add pass.
x is read once from HBM and the output written once — memory-roofline shape.

Per core (4096 rows): for each 128-row tile, PE transposes the tile
(8 x 128x128, via identity matmul) into PSUM, ACT copies it back to SBUF,
PE accumulates C = xT^T @ W^T chunks into PSUM, DVE runs the alpha
recurrence and the single fused out = (x0 * alpha) + beta pass.

Sharding: batch dim of x split across the 8 cores; the tiny (L,D)-derived
tensors (W^T chunks, beta_L, gammas) are replicated.
"""

import numpy as np

import concourse.bass as bass
import concourse.tile as tile
from concourse import mybir
from concourse.bass_utils import run_bass_kernel_spmd
from concourse.masks import make_identity
from concourse.vector_clock import ScopedClock

F32 = mybir.dt.float32
AL = mybir.AluOpType

B, D, L = 32768, 1024, 4
N_CORES = 8
BC = B // N_CORES          # rows per core
P = 128                    # SBUF partitions
NCHUNK = D // P            # 8 column chunks of 128
NT = BC // P               # 32 row-tiles per core

# Engine split for the C = x @ W^T dot products: PE handles d-chunks
# [0, K_PE*128) via transpose+matmul; DVE handles the tail d-range via
# fused multiply-reduce (tensor_tensor_reduce). Balances PE vs DVE so both
# hide under the ~95us/core HBM floor.
K_PE = NCHUNK - 3          # 5 chunks on PE
D_PE = K_PE * P            # 768
D_DVE = D - D_PE           # 256


class SplitDrainTileContext(tile.TileContext):
    """The walrus build in this container rejects >4 sync waits on a single
    instruction, but the stock kernel-tail drain funnels every outstanding
    proc's wait onto one SP Drain. Redistribute them into a chain of
    single-wait drains (semantically identical: SP waits for each proc in
    turn before the exit barrier)."""

    MAXW = 1

    def _drain_and_barrier(self, tick_clock, wait_clock):
        drain_inst = self.nc.sync.drain()
        wait_clock.add_sem_waits(
            drain_inst.ins, ScopedClock({None: tick_clock.global_clock})
        )
        si = drain_inst.ins.sync_info
        waits = list(si.on_wait) if si is not None and si.on_wait else []
        if len(waits) > self.MAXW:
            drain_inst.ins.sync_info = mybir.SyncInfo(
                on_wait=waits[: self.MAXW],
                on_update=list(si.on_update or []),
            )
            rest = waits[self.MAXW:]
            for i in range(0, len(rest), self.MAXW):
                d2 = self.nc.sync.drain()
                d2.ins.sync_info = mybir.SyncInfo(
                    on_wait=rest[i : i + self.MAXW], on_update=[]
                )
        self.nc.all_engine_barrier()
        assert self.sems is not None
        popped = self.nc._tile_sem_poison_stack.pop()
        assert popped is self._sem_poison
        self.nc.clear_and_free_semaphores(list(self.sems.allocated().values()))
        self.nc.all_engine_barrier()


def _split_multiwait_insts(nc, maxw=1):
    """Walrus here rejects instructions carrying more than a few sync waits.
    Hoist excess waits onto single-wait NOPs inserted just before the
    offending instruction on the same engine (identical blocking
    semantics: the engine waits on each sem in turn)."""
    for bb in nc.main_func.blocks:
        insts = list(bb.bb.instructions if hasattr(bb, "bb") else bb.instructions)
        changed = False
        new = []
        for ins in insts:
            si = getattr(ins, "sync_info", None)
            waits = list(si.on_wait) if si is not None and si.on_wait else []
            if len(waits) > maxw and ins.engine != mybir.EngineType.Unassigned:
                extra, keep = waits[:-maxw], waits[-maxw:]
                for k in range(0, len(extra), maxw):
                    nop = mybir.InstNoOp(
                        name=nc.get_next_instruction_name(), ins=[], outs=[]
                    )
                    nop.engine = ins.engine
                    nop.sync_info = mybir.SyncInfo(
                        on_wait=extra[k : k + maxw], on_update=[]
                    )
                    new.append(nop)
                ins.sync_info = mybir.SyncInfo(
                    on_wait=keep, on_update=list(si.on_update or [])
                )
                changed = True
            new.append(ins)
        if changed:
            container = bb.bb if hasattr(bb, "bb") else bb
            container.instructions.clear()
            for ins in new:
                container.instructions.append(ins)


def build_kernel(repeat=1, body_passes=1):
    """repeat>1 wraps the whole tile loop in a dynamic For_i that re-runs it
    (same inputs/outputs) -- used only for on-device timing, where wall-clock
    differencing of two repeat counts cancels dispatch/transfer overhead.
    body_passes replicates the tile loop inside the For_i body so the loop
    back-edge cost can be cancelled out too."""
    nc = bass.Bass(target_bir_lowering=False)
    x_d = nc.dram_tensor("x", [BC, D], F32, kind="ExternalInput")
    # wt[p, j, l] = W[l, 128*j + p]  (host-pretransposed W^T, chunked)
    wt_d = nc.dram_tensor("wt", [P, K_PE, L], F32, kind="ExternalInput")
    # wb[0, l*D_DVE + m] = W[l, D_PE + m]  (tail chunks, row-broadcast)
    wb_d = nc.dram_tensor("wb", [1, L * D_DVE], F32, kind="ExternalInput")
    beta_d = nc.dram_tensor("beta", [1, D], F32, kind="ExternalInput")
    gam_d = nc.dram_tensor("gam", [1, L], F32, kind="ExternalInput")
    out_d = nc.dram_tensor("out", [BC, D], F32, kind="ExternalOutput")

    with SplitDrainTileContext(nc) as tc:
        with (
            tc.tile_pool(name="consts", bufs=1) as consts,
            tc.tile_pool(name="xp", bufs=4) as xp,
            tc.tile_pool(name="xtp", bufs=5) as xtp,
            tc.tile_pool(name="op", bufs=3) as op,
            tc.tile_pool(name="small", bufs=8) as small,
            tc.tile_pool(name="pst", bufs=3, space="PSUM") as pst,
            tc.tile_pool(name="psc", bufs=2, space="PSUM") as psc,
        ):
            wt_sb = consts.tile([P, K_PE, L], F32)
            nc.sync.dma_start(wt_sb[:], wt_d[:, :, :])
            wb_sb = consts.tile([P, L * D_DVE], F32)
            nc.gpsimd.dma_start(
                wb_sb[:], wb_d[:, :].to_broadcast((P, L * D_DVE))
            )
            beta_sb = consts.tile([P, D], F32)
            nc.gpsimd.dma_start(beta_sb[:], beta_d[:, :].to_broadcast((P, D)))
            gam_sb = consts.tile([P, L], F32)
            nc.gpsimd.dma_start(gam_sb[:], gam_d[:, :].to_broadcast((P, L)))
            ident = consts.tile([P, P], F32)
            make_identity(nc, ident)

            import contextlib

            rep_ctx = (
                tc.For_i(0, repeat, 1) if repeat > 1 else contextlib.nullcontext()
            )
            with rep_ctx:
                for _ in range(body_passes):
                    _tile_loop(nc, tc, x_d, out_d, wt_sb, wb_sb, beta_sb,
                               gam_sb, ident, xp, xtp, op, small, pst, psc)
    _split_multiwait_insts(nc)
    return nc


ST = 2                     # tiles per supertile: 1 MiB per DMA (max-BW knee)
NST = NT // ST


def _tile_loop(nc, tc, x_d, out_d, wt_sb, wb_sb, beta_sb, gam_sb, ident,
               xp, xtp, op, small, pst, psc):
    """Supertiles of ST row-tiles: one 1MiB load (SP-issued HWDGE) and one
    1MiB store (ACT-issued HWDGE) each — the two issuers use different DMA
    queue sets, together reaching ~335 GB/s/core vs ~228 single-issuer.
    One-stage-skewed pipeline so PE's matmuls (which wait on ACT's
    PSUM->SBUF copy) always have next-supertile transposes behind them."""
    state = {}

    def stage_a(u):
        x_sb = xp.tile([P, ST, D], F32)
        src = x_d[u * ST * P:(u + 1) * ST * P, :].rearrange(
            "(s p) d -> p s d", p=P
        )
        nc.sync.dma_start(x_sb[:], src)

        subs = []
        for s in range(ST):
            xs = x_sb[:, s, :]
            # head chunks transposed on PE -> PSUM, ACT copies back to SBUF
            xt_ps = pst.tile([P, K_PE, P], F32)
            for j in range(K_PE):
                nc.tensor.transpose(
                    xt_ps[:, j, :], xs[:, j * P:(j + 1) * P], ident
                )
            xt_sb = xtp.tile([P, K_PE, P], F32)
            nc.scalar.copy(xt_sb[:], xt_ps[:])

            # DVE tail dot (independent of PE): fused multiply with
            # free-axis sum into ct_sb (one pass per layer)
            ct_sb = small.tile([P, L], F32)
            prod = xtp.tile([P, D_DVE], F32, tag="prod")
            for l in range(L):
                nc.vector.scalar_tensor_tensor(
                    out=prod[:],
                    in0=xs[:, D_PE:],
                    scalar=1.0,
                    in1=wb_sb[:, l * D_DVE:(l + 1) * D_DVE],
                    op0=AL.mult,
                    op1=AL.mult,
                    accum_out=ct_sb[:, l:l + 1],
                )
            subs.append((xt_sb, ct_sb))
        state[u] = (x_sb, subs)

    def stage_b(u):
        x_sb, subs = state.pop(u)
        o_sb = op.tile([P, ST, D], F32)
        for s in range(ST):
            xt_sb, ct_sb = subs[s]
            # PE partial dot: c_pe[r, l] = sum_{d < D_PE} x[r, d] W[l, d]
            c_ps = psc.tile([P, L], F32)
            for j in range(K_PE):
                nc.tensor.matmul(
                    c_ps[:],
                    xt_sb[:, j, :],
                    wt_sb[:, j, :],
                    start=(j == 0),
                    stop=(j == K_PE - 1),
                )

            # T_i = 1 + c_i with c_i = c_pe + c_tail (c_pe read straight
            # from PSUM), fused in one tiny op
            t_sb = small.tile([P, L], F32)
            nc.vector.scalar_tensor_tensor(
                out=t_sb[:],
                in0=c_ps[:],
                scalar=1.0,
                in1=ct_sb[:],
                op0=AL.add,
                op1=AL.add,
            )
            # whole alpha recurrence in one scan:
            # state_{i+1} = (T_i * state_i) + gamma_i, state_0 = 1
            al_sb = small.tile([P, L], F32)
            nc.vector.tensor_tensor_scan(
                out=al_sb[:],
                data0=t_sb[:],
                data1=gam_sb[:],
                initial=1.0,
                op0=AL.mult,
                op1=AL.add,
            )

            # out = alpha_L * x0 + beta_L, one fused DVE pass
            nc.vector.scalar_tensor_tensor(
                out=o_sb[:, s, :],
                in0=x_sb[:, s, :],
                scalar=al_sb[:, L - 1:L],
                in1=beta_sb[:],
                op0=AL.mult,
                op1=AL.add,
            )
        dst = out_d[u * ST * P:(u + 1) * ST * P, :].rearrange(
            "(s p) d -> p s d", p=P
        )
        nc.scalar.dma_start(dst, o_sb[:])

    for u in range(NST + 1):
        if u < NST:
            stage_a(u)
        if u >= 1:
            stage_b(u - 1)


_NC_CACHE = []


def _get_nc():
    if not _NC_CACHE:
        _NC_CACHE.append(build_kernel())
    return _NC_CACHE[0]


def prep_inputs(x, weights, biases):
    """Shard x by batch across cores; derive the tiny replicated tensors."""
    x = np.ascontiguousarray(np.asarray(x, dtype=np.float32))
    w = np.asarray(weights, dtype=np.float64)
    b = np.asarray(biases, dtype=np.float64)
    assert x.shape == (B, D) and w.shape == (L, D) and b.shape == (L, D)

    betas = np.concatenate([np.zeros((1, D)), np.cumsum(b, axis=0)], axis=0)
    gammas = np.array([betas[i] @ w[i] for i in range(L)])  # gamma_0 = 0
    beta_l = betas[L].astype(np.float32)[None, :]
    gam = gammas.astype(np.float32)[None, :]
    wf = w.astype(np.float32)
    # wt[p, j, l] = W[l, 128*j + p] for the PE chunks
    wt = np.ascontiguousarray(
        wf[:, :D_PE].T.reshape(K_PE, P, L).transpose(1, 0, 2)
    )
    # wb[0, l*D_DVE + m] = W[l, D_PE + m] for the DVE tail
    wb = np.ascontiguousarray(wf[:, D_PE:].reshape(1, L * D_DVE))
    in_maps = [
        {
            "x": x[c * BC:(c + 1) * BC],
            "wt": wt,
            "wb": wb,
            "beta": beta_l,
            "gam": gam,
        }
        for c in range(N_CORES)
    ]
    return in_maps


def run_sharded(x, weights, biases, **run_kwargs):
    nc = _get_nc()
    in_maps = prep_inputs(x, weights, biases)
    res = run_bass_kernel_spmd(nc, in_maps, core_ids=list(range(N_CORES)), **run_kwargs)
    out = np.concatenate([r["out"] for r in res.results], axis=0)
    return out, res


def kernel(x, weights, biases):
    out, _ = run_sharded(x, weights, biases)
    return out


def build_dma_only(repeat=1, body_passes=1):
    """Diagnostic: just stream x in and out, no compute."""
    nc = bass.Bass(target_bir_lowering=False)
    x_d = nc.dram_tensor("x", [BC, D], F32, kind="ExternalInput")
    out_d = nc.dram_tensor("out", [BC, D], F32, kind="ExternalOutput")
    with SplitDrainTileContext(nc) as tc:
        with tc.tile_pool(name="xp", bufs=8) as xp:
            import contextlib
            rep_ctx = (
                tc.For_i(0, repeat, 1) if repeat > 1 else contextlib.nullcontext()
            )
            with rep_ctx:
                for _ in range(body_passes):
                    for t in range(NT // 2):
                        x_sb = xp.tile([P, 2, D], F32)
                        src = x_d[t * 2 * P:(t + 1) * 2 * P, :].rearrange(
                            "(s p) d -> p s d", p=P)
                        ldeng = nc.sync if t % 2 == 0 else nc.scalar
                        steng = nc.scalar if t % 2 == 0 else nc.sync
                        ldeng.dma_start(x_sb[:], src)
                        dst = out_d[t * 2 * P:(t + 1) * 2 * P, :].rearrange(
                            "(s p) d -> p s d", p=P)
                        steng.dma_start(dst, x_sb[:])
    _split_multiwait_insts(nc)
    return nc



# revision 25
# speedup vs baseline: 127768.6436x; 127768.6436x over previous
"""DCN cross-layer kernel for Trainium2 (8 NeuronCores, data-parallel).

Reference computes, for i = 0..L-1:
    x_{i+1} = x0 * (x_i . w_i) + b_i + x_i         (x0 fixed, per-row dot)

Algebraic collapse: every iterate has the form x_i = alpha_i * x0 + beta_i
with per-row scalar alpha_i and a row-independent vector beta_i:
    alpha_0 = 1,  beta_0 = 0
    alpha_{i+1} = alpha_i * (1 + c_i) + gamma_i,   c_i = x0 . w_i (per row)
    beta_{i+1}  = beta_i + b_i,                    gamma_i = beta_i . w_i
    out = alpha_L * x0 + beta_L

So the whole module reduces to one skinny matmul C = x0 @ W^T (B x L), a
tiny per-row recurrence over L=4 scalars, and one fused scale-add pass.
x is read once from HBM and the output written once — memory-roofline shape.

bf16 streaming: x is pre-rounded to bf16 on the host and the output is
produced in bf16 (upconverted to fp32 on the host), halving HBM traffic
vs fp32 (16.8 MB/core instead of 33.6 MB). All on-device accumulation is
fp32 (PE PSUM + DVE internal); measured end-to-end relative error is
~4e-3, well inside the 2e-2 gate.

Per core (4096 rows): for each 128-row tile, PE transposes the K_PE head
chunks (bf16, 1 cyc/row) into PSUM, ACT copies them back to SBUF as bf16,
PE accumulates C = xT^T @ W^T chunks into PSUM; any tail chunks are dotted
on DVE via scalar_tensor_tensor+accum. ACT computes t = 1 + c straight
from PSUM, DVE runs the alpha scan, then the out-pass is two accelerated
DVE ops: o = x * alpha (tensor_scalar, 4x mode) and o += beta
(tensor_tensor, 2x mode) — the fused 1x STT would be slower than the pair.

Sharding: batch dim of x split across the 8 cores; the tiny (L,D)-derived
tensors (W^T chunks, beta_L, gammas) are replicated.
"""

import numpy as np

import concourse.bass as bass
import concourse.tile as tile
from concourse import mybir
from concourse.bass_utils import run_bass_kernel_spmd
from concourse.masks import make_identity
from concourse.vector_clock import ScopedClock

F32 = mybir.dt.float32
BF16 = mybir.dt.bfloat16
AL = mybir.AluOpType
AF = mybir.ActivationFunctionType

B, D, L = 32768, 1024, 4
N_CORES = 8
BC = B // N_CORES          # rows per core
P = 128                    # SBUF partitions
NCHUNK = D // P            # 8 column chunks of 128
NT = BC // P               # 32 row-tiles per core

# Engine split for the C = x @ W^T dot products: PE handles d-chunks
# [0, K_PE*128) via transpose+matmul (bf16: 1 cyc/row); DVE handles any
# tail via fused multiply-reduce (1x mode only — keep the tail small).
K_PE = 8                   # all chunks on PE by default
D_PE = K_PE * P
D_DVE = D - D_PE

ST = 4                     # tiles per supertile: 1 MiB per DMA (max-BW knee)
NST = NT // ST

# shipped configuration (build_kernel params exist for dev experiments)
STORE_ENG = "scalar"       # loads on SP ring, stores on ACT ring
COPY_DVE = 0               # PSUM->SBUF copy entirely on ACT
ABLATE = ""
LATE_STORE = False


class SplitDrainTileContext(tile.TileContext):
    """The walrus build in this container rejects >4 sync waits on a single
    instruction, but the stock kernel-tail drain funnels every outstanding
    proc's wait onto one SP Drain. Redistribute them into a chain of
    single-wait drains (semantically identical: SP waits for each proc in
    turn before the exit barrier)."""

    MAXW = 1

    def _drain_and_barrier(self, tick_clock, wait_clock):
        drain_inst = self.nc.sync.drain()
        wait_clock.add_sem_waits(
            drain_inst.ins, ScopedClock({None: tick_clock.global_clock})
        )
        si = drain_inst.ins.sync_info
        waits = list(si.on_wait) if si is not None and si.on_wait else []
        if len(waits) > self.MAXW:
            drain_inst.ins.sync_info = mybir.SyncInfo(
                on_wait=waits[: self.MAXW],
                on_update=list(si.on_update or []),
            )
            rest = waits[self.MAXW:]
            for i in range(0, len(rest), self.MAXW):
                d2 = self.nc.sync.drain()
                d2.ins.sync_info = mybir.SyncInfo(
                    on_wait=rest[i : i + self.MAXW], on_update=[]
                )
        self.nc.all_engine_barrier()
        assert self.sems is not None
        popped = self.nc._tile_sem_poison_stack.pop()
        assert popped is self._sem_poison
        self.nc.clear_and_free_semaphores(list(self.sems.allocated().values()))
        self.nc.all_engine_barrier()


def _split_multiwait_insts(nc, maxw=1):
    """Walrus here rejects instructions carrying more than a few sync waits.
    Hoist excess waits onto single-wait NOPs inserted just before the
    offending instruction on the same engine (identical blocking
    semantics: the engine waits on each sem in turn)."""
    for bb in nc.main_func.blocks:
        insts = list(bb.bb.instructions if hasattr(bb, "bb") else bb.instructions)
        changed = False
        new = []
        for ins in insts:
            si = getattr(ins, "sync_info", None)
            waits = list(si.on_wait) if si is not None and si.on_wait else []
            if len(waits) > maxw and ins.engine != mybir.EngineType.Unassigned:
                extra, keep = waits[:-maxw], waits[-maxw:]
                for k in range(0, len(extra), maxw):
                    nop = mybir.InstNoOp(
                        name=nc.get_next_instruction_name(), ins=[], outs=[]
                    )
                    nop.engine = ins.engine
                    nop.sync_info = mybir.SyncInfo(
                        on_wait=extra[k : k + maxw], on_update=[]
                    )
                    new.append(nop)
                ins.sync_info = mybir.SyncInfo(
                    on_wait=keep, on_update=list(si.on_update or [])
                )
                changed = True
            new.append(ins)
        if changed:
            container = bb.bb if hasattr(bb, "bb") else bb
            container.instructions.clear()
            for ins in new:
                container.instructions.append(ins)


def build_kernel(repeat=1, body_passes=1, store_eng=None, copy_dve=None,
                 ablate=None, st=None, late_store=None):
    """repeat>1 wraps the whole tile loop in a dynamic For_i that re-runs it
    (same inputs/outputs) -- used only for on-device timing, where wall-clock
    differencing of two repeat counts cancels dispatch/transfer overhead.
    body_passes replicates the tile loop inside the For_i body so the loop
    back-edge cost can be cancelled out too."""
    cfg = dict(
        store_eng=STORE_ENG if store_eng is None else store_eng,
        copy_dve=COPY_DVE if copy_dve is None else copy_dve,
        ablate=ABLATE if ablate is None else ablate,
        st=ST if st is None else st,
        late_store=LATE_STORE if late_store is None else late_store,
    )
    nc = bass.Bass(target_bir_lowering=False)
    x_d = nc.dram_tensor("x", [BC, D], BF16, kind="ExternalInput")
    # wt[p, j, l] = W[l, 128*j + p]  (host-pretransposed W^T, chunked)
    wt_d = nc.dram_tensor("wt", [P, K_PE, L], BF16, kind="ExternalInput")
    if D_DVE:
        # wb[0, l*D_DVE + m] = W[l, D_PE + m]  (tail chunks, row-broadcast)
        wb_d = nc.dram_tensor("wb", [1, L * D_DVE], BF16, kind="ExternalInput")
    beta_d = nc.dram_tensor("beta", [1, D], BF16, kind="ExternalInput")
    gam_d = nc.dram_tensor("gam", [1, L], F32, kind="ExternalInput")
    out_d = nc.dram_tensor("out", [BC, D], BF16, kind="ExternalOutput")

    with SplitDrainTileContext(nc) as tc:
        with (
            tc.tile_pool(name="consts", bufs=1) as consts,
            tc.tile_pool(name="xp", bufs=4) as xp,
            tc.tile_pool(name="xtp", bufs=5) as xtp,
            tc.tile_pool(name="op", bufs=4) as op,
            tc.tile_pool(name="small", bufs=8) as small,
            tc.tile_pool(name="pst", bufs=3, space="PSUM") as pst,
            tc.tile_pool(name="psc", bufs=2, space="PSUM") as psc,
        ):
            wt_sb = consts.tile([P, K_PE, L], BF16)
            nc.sync.dma_start(wt_sb[:], wt_d[:, :, :])
            if D_DVE:
                wb_sb = consts.tile([P, L * D_DVE], BF16)
                nc.gpsimd.dma_start(
                    wb_sb[:], wb_d[:, :].to_broadcast((P, L * D_DVE))
                )
            else:
                wb_sb = None
            beta_sb = consts.tile([P, D], BF16)
            nc.gpsimd.dma_start(beta_sb[:], beta_d[:, :].to_broadcast((P, D)))
            gam_sb = consts.tile([P, L], F32)
            nc.gpsimd.dma_start(gam_sb[:], gam_d[:, :].to_broadcast((P, L)))
            ident = consts.tile([P, P], BF16)
            make_identity(nc, ident)

            import contextlib

            rep_ctx = (
                tc.For_i(0, repeat, 1) if repeat > 1 else contextlib.nullcontext()
            )
            with rep_ctx:
                for _ in range(body_passes):
                    _tile_loop(nc, tc, x_d, out_d, wt_sb, wb_sb, beta_sb,
                               gam_sb, ident, xp, xtp, op, small, pst, psc,
                               cfg)
    _split_multiwait_insts(nc)
    return nc


def _tile_loop(nc, tc, x_d, out_d, wt_sb, wb_sb, beta_sb, gam_sb, ident,
               xp, xtp, op, small, pst, psc, cfg):
    """Supertiles of ST row-tiles: one 1MiB load (SP-issued HWDGE) and one
    1MiB store (ACT-issued HWDGE) each — the two issuers use different DMA
    queue sets. One-stage-skewed pipeline so PE's matmuls (which wait on
    ACT's PSUM->SBUF copy) always have next-supertile transposes behind
    them."""
    state = {}
    ostate = {}
    st = cfg["st"]
    nst = NT // st
    copy_dve = cfg["copy_dve"]
    ablate = cfg["ablate"]
    store_cfg = cfg["store_eng"]
    late = cfg["late_store"]

    def stage_a(u):
        x_sb = xp.tile([P, st, D], BF16)
        src = x_d[u * st * P:(u + 1) * st * P, :].rearrange(
            "(s p) d -> p s d", p=P
        )
        ldeng = (nc.sync if u % 2 == 0 else nc.scalar) \
            if store_cfg == "alt" else nc.sync
        ldeng.dma_start(x_sb[:], src)

        subs = []
        for s in range(st):
            xs = x_sb[:, s, :]
            if ablate == "nodots":
                subs.append((None, None))
                continue
            # head chunks transposed on PE -> PSUM, ACT copies back to SBUF
            kact = K_PE - copy_dve
            xt_ps = pst.tile([P, kact, P], BF16)
            for j in range(kact):
                nc.tensor.transpose(
                    xt_ps[:, j, :], xs[:, j * P:(j + 1) * P], ident
                )
            if copy_dve:
                xt_ps2 = pst.tile([P, copy_dve, P], BF16, tag="ps2")
                for j in range(copy_dve):
                    nc.tensor.transpose(
                        xt_ps2[:, j, :],
                        xs[:, (kact + j) * P:(kact + j + 1) * P], ident
                    )
            xt_sb = xtp.tile([P, K_PE, P], BF16)
            if copy_dve:
                nc.scalar.copy(xt_sb[:, :kact], xt_ps[:])
                nc.vector.tensor_copy(xt_sb[:, kact:], xt_ps2[:])
            else:
                nc.scalar.copy(xt_sb[:], xt_ps[:])

            if D_DVE:
                # DVE tail dot (independent of PE): fused multiply with
                # free-axis sum into ct_sb (one pass per layer)
                ct_sb = small.tile([P, L], F32)
                prod = xtp.tile([P, D_DVE], BF16, tag="prod")
                for l in range(L):
                    nc.vector.scalar_tensor_tensor(
                        out=prod[:],
                        in0=xs[:, D_PE:],
                        scalar=1.0,
                        in1=wb_sb[:, l * D_DVE:(l + 1) * D_DVE],
                        op0=AL.mult,
                        op1=AL.mult,
                        accum_out=ct_sb[:, l:l + 1],
                    )
            else:
                ct_sb = None
            subs.append((xt_sb, ct_sb))
        state[u] = (x_sb, subs)

    def stage_b(u):
        x_sb, subs = state.pop(u)
        o_sb = op.tile([P, st, D], BF16)
        for s in range(st):
            xt_sb, ct_sb = subs[s]
            if ablate == "nodots":
                nc.vector.tensor_scalar_mul(o_sb[:, s, :], x_sb[:, s, :], 2.0)
                nc.vector.tensor_tensor(
                    out=o_sb[:, s, :], in0=o_sb[:, s, :], in1=beta_sb[:],
                    op=AL.add,
                )
                continue
            # PE partial dot: c[r, l] = sum_{d < D_PE} x[r, d] W[l, d]
            c_ps = psc.tile([P, L], F32)
            for j in range(K_PE):
                nc.tensor.matmul(
                    c_ps[:],
                    xt_sb[:, j, :],
                    wt_sb[:, j, :],
                    start=(j == 0),
                    stop=(j == K_PE - 1),
                )

            t_sb = small.tile([P, L], F32)
            if D_DVE:
                # T_i = 1 + c_pe + c_tail, fused in one tiny DVE op
                nc.vector.scalar_tensor_tensor(
                    out=t_sb[:],
                    in0=c_ps[:],
                    scalar=1.0,
                    in1=ct_sb[:],
                    op0=AL.add,
                    op1=AL.add,
                )
            else:
                # T_i = 1 + c, on ACT (sits next to PSUM; frees DVE)
                nc.scalar.activation(
                    t_sb[:], c_ps[:], AF.Identity, bias=1.0
                )
            # whole alpha recurrence in one scan:
            # state_{i+1} = (T_i * state_i) + gamma_i, state_0 = 1
            al_sb = small.tile([P, L], F32)
            nc.vector.tensor_tensor_scan(
                out=al_sb[:],
                data0=t_sb[:],
                data1=gam_sb[:],
                initial=1.0,
                op0=AL.mult,
                op1=AL.add,
            )

            if ablate == "noout":
                continue
            # out = alpha_L * x0 + beta_L as two accelerated DVE passes
            # (tensor_scalar runs 4x on bf16, tensor_tensor 2x; the fused
            # STT form would run 1x and be slower than both together)
            nc.vector.tensor_scalar_mul(
                o_sb[:, s, :], x_sb[:, s, :], al_sb[:, L - 1:L]
            )
            nc.vector.tensor_tensor(
                out=o_sb[:, s, :],
                in0=o_sb[:, s, :],
                in1=beta_sb[:],
                op=AL.add,
            )
        if late and u < nst - 1:
            ostate[u] = o_sb
        else:
            stage_c(u, o_sb)

    def stage_c(u, o_sb):
        dst = out_d[u * st * P:(u + 1) * st * P, :].rearrange(
            "(s p) d -> p s d", p=P
        )
        if store_cfg == "alt":
            store_eng = nc.scalar if u % 2 == 0 else nc.sync
        else:
            store_eng = getattr(nc, store_cfg)
        store_eng.dma_start(dst, o_sb[:])

    # late: stores issued one slot after compute, so the issuing engine's
    # wait-for-DVE has already been satisfied and it never stalls
    for u in range(nst + 2):
        if u < nst:
            stage_a(u)
        if late and u >= 2 and (u - 2) in ostate:
            stage_c(u - 2, ostate.pop(u - 2))
        if 1 <= u <= nst:
            stage_b(u - 1)


_NC_CACHE = {}


def _get_nc(repeat=1, body_passes=1):
    key = (repeat, body_passes)
    if key not in _NC_CACHE:
        _NC_CACHE[key] = build_kernel(repeat, body_passes)
    return _NC_CACHE[key]


def _to_bf16(a):
    import ml_dtypes
    return np.asarray(a).astype(ml_dtypes.bfloat16)


def prep_inputs(x, weights, biases):
    """Shard x by batch across cores; derive the tiny replicated tensors."""
    x = np.ascontiguousarray(np.asarray(x, dtype=np.float32))
    w = np.asarray(weights, dtype=np.float64)
    b = np.asarray(biases, dtype=np.float64)
    assert x.shape == (B, D) and w.shape == (L, D) and b.shape == (L, D)

    betas = np.concatenate([np.zeros((1, D)), np.cumsum(b, axis=0)], axis=0)
    gammas = np.array([betas[i] @ w[i] for i in range(L)])  # gamma_0 = 0
    beta_l = _to_bf16(betas[L])[None, :]
    gam = gammas.astype(np.float32)[None, :]
    wf = w.astype(np.float32)
    # wt[p, j, l] = W[l, 128*j + p] for the PE chunks
    wt = _to_bf16(np.ascontiguousarray(
        wf[:, :D_PE].T.reshape(K_PE, P, L).transpose(1, 0, 2)
    ))
    xb = _to_bf16(x)
    in_maps = []
    for c in range(N_CORES):
        m = {
            "x": np.ascontiguousarray(xb[c * BC:(c + 1) * BC]),
            "wt": wt,
            "beta": beta_l,
            "gam": gam,
        }
        if D_DVE:
            # wb[0, l*D_DVE + m] = W[l, D_PE + m] for the DVE tail
            m["wb"] = _to_bf16(
                np.ascontiguousarray(wf[:, D_PE:].reshape(1, L * D_DVE))
            )
        in_maps.append(m)
    return in_maps


def run_sharded(x, weights, biases, repeat=1, body_passes=1, **run_kwargs):
    nc = _get_nc(repeat, body_passes)
    in_maps = prep_inputs(x, weights, biases)
    res = run_bass_kernel_spmd(nc, in_maps, core_ids=list(range(N_CORES)), **run_kwargs)
    out = np.concatenate(
        [np.asarray(r["out"]).astype(np.float32) for r in res.results], axis=0
    )
    return out, res


def kernel(x, weights, biases):
    out, _ = run_sharded(x, weights, biases)
    return out


def build_dma_only(repeat=1, body_passes=1):
    """Diagnostic: just stream x in and out, no compute."""
    nc = bass.Bass(target_bir_lowering=False)
    x_d = nc.dram_tensor("x", [BC, D], BF16, kind="ExternalInput")
    out_d = nc.dram_tensor("out", [BC, D], BF16, kind="ExternalOutput")
    with SplitDrainTileContext(nc) as tc:
        with tc.tile_pool(name="xp", bufs=8) as xp:
            import contextlib
            rep_ctx = (
                tc.For_i(0, repeat, 1) if repeat > 1 else contextlib.nullcontext()
            )
            with rep_ctx:
                for _ in range(body_passes):
                    for t in range(NT // ST):
                        x_sb = xp.tile([P, ST, D], BF16)
                        src = x_d[t * ST * P:(t + 1) * ST * P, :].rearrange(
                            "(s p) d -> p s d", p=P)
                        ldeng = nc.sync if t % 2 == 0 else nc.scalar
                        steng = nc.scalar if t % 2 == 0 else nc.sync
                        ldeng.dma_start(x_sb[:], src)
                        dst = out_d[t * ST * P:(t + 1) * ST * P, :].rearrange(
                            "(s p) d -> p s d", p=P)
                        steng.dma_start(dst, x_sb[:])
    _split_multiwait_insts(nc)
    return nc
